# revision 13
# baseline (speedup 1.0000x reference)
"""Trainium2 Bass kernel for channel (cross-covariance) self-attention.

Shapes (hardcoded): x (8, 4096, 512) f32, wqkv_w (1536, 512), wqkv_b (1536,),
wp_w (512, 512), wp_b (512,). NUM_HEADS=8, head_dim=64.

Sharding: data-parallel over batch b across the 8 NeuronCores (one batch
element per core). Weights replicated.

Per-core algorithm. The reference's (b,n,h,d)->(b,h,n,d) reshape is a flat
reinterpretation, so head h's (4096, 64) q/k/v matrix is the contiguous
(512, 512) token-block rows [h*512,(h+1)*512) of the (4096, c) q/k/v matrix
reinterpreted: Q_h[8t+s, d] = q[h*512+t, s*64+d]. Hence:
  - scores_h[d,e] = sum_{t,s} q[h512+t, s64+d] * k[h512+t, s64+e]
    -> 32 PE matmuls (K=128 over t, accumulated over 4 t-tiles x 8 s) per head.
  - y_h[d, m=8t+s] = sum_e W_h[d,e] v[h512+t, s64+e]
    -> matmul with block-diag(W_h^T) stationary against vT tiles.
  - output token m=8t+s channel h*64+d = y_h[d, m]; assembled transposed
    (channel-partition) for the output projection.
All big matmuls run in float32r (tf32-class, 1 cyc/row at N=512); the tiny
per-head scores matmuls run in plain fp32 (same cost at N=64, full precision).
"""

import numpy as np
from contextlib import ExitStack

import concourse.bass as bass
import concourse.tile as tile
from concourse import bacc, mybir
from concourse.bass_utils import run_bass_kernel_spmd
from concourse.masks import make_identity

dt = mybir.dt

N_TOK = 4096
C = 512
H = 8
D = 64
TB = 512          # tokens per head block
SUB = TB // 128   # 4 t-tiles per head block
CT = C // 128     # 4 contraction tiles

_cache = {}


def _emit(ctx: ExitStack, tc, out_d, x_d, wqkv_d, wqkvb_d, wp_d, wpb_d):
    nc = tc.nc
    f32, f32r = dt.float32, dt.float32r

    const = ctx.enter_context(tc.tile_pool(name="const", bufs=1))
    smallp = ctx.enter_context(tc.tile_pool(name="smallp", bufs=2))
    outp = ctx.enter_context(tc.tile_pool(name="outp", bufs=3))
    ps_big = ctx.enter_context(tc.tile_pool(name="ps_big", bufs=4, space="PSUM"))
    ps_sm = ctx.enter_context(tc.tile_pool(name="ps_sm", bufs=2, space="PSUM"))

    # ---------------- one-time setup ----------------
    ident = const.tile([128, 128], f32)
    make_identity(nc, ident)
    zeros128 = const.tile([128, 128], f32)
    nc.vector.memset(zeros128, 0.0)

    # Persistent setup outputs.
    wqkvT = const.tile([128, CT, 3 * C], f32r)
    wpT = const.tile([128, CT, C], f32r)
    qk_bias = const.tile([128, 2 * C], f32)
    vbias = const.tile([128, CT], f32)
    wp_bias = const.tile([128, C], f32)
    # Transposed attention output, channel-major: attnT[p, j, m] = attn_out[m, j*128+p]
    attnT = const.tile([128, CT, N_TOK], f32r)

    with tc.tile_pool(name="setup", bufs=3) as setup:
        # Transposed weights: WqkvT[p, ct, f] = wqkv_w[f, ct*128+p]
        for wt in range(3 * C // 128):          # 12 feature tiles
            st = setup.tile([128, C], f32, tag="wstage")
            nc.sync.dma_start(st, wqkv_d[wt * 128:(wt + 1) * 128, :])
            for ct in range(CT):
                tp = ps_sm.tile([128, 128], f32, tag="pss")
                nc.tensor.transpose(tp, st[:, ct * 128:(ct + 1) * 128], ident)
                nc.vector.tensor_copy(wqkvT[:, ct, wt * 128:(wt + 1) * 128], tp)

        # WpT[p, j, f] = wp_w[f, j*128+p]
        for wt in range(C // 128):              # 4 feature tiles
            st = setup.tile([128, C], f32, tag="wstage")
            nc.sync.dma_start(st, wp_d[wt * 128:(wt + 1) * 128, :])
            for j in range(CT):
                tp = ps_sm.tile([128, 128], f32, tag="pss")
                nc.tensor.transpose(tp, st[:, j * 128:(j + 1) * 128], ident)
                nc.vector.tensor_copy(wpT[:, j, wt * 128:(wt + 1) * 128], tp)

        # Biases.
        qkb_row = setup.tile([1, 3 * C], f32, tag="brow")
        nc.sync.dma_start(qkb_row, wqkvb_d.rearrange("(a f) -> a f", a=1))
        nc.gpsimd.partition_broadcast(qk_bias, qkb_row[0:1, 0:2 * C])
        # v bias, per-partition: vbias[p, ct] = wqkv_b[1024 + ct*128 + p]
        nc.sync.dma_start(vbias, wqkvb_d[2 * C:3 * C].rearrange("(ct p) -> p ct", p=128))
        wpb_row = setup.tile([1, C], f32, tag="brow2")
        nc.sync.dma_start(wpb_row, wpb_d.rearrange("(a f) -> a f", a=1))
        nc.gpsimd.partition_broadcast(wp_bias, wpb_row[0:1, :])

    stage = ctx.enter_context(tc.tile_pool(name="stage", bufs=6))
    headp = ctx.enter_context(tc.tile_pool(name="headp", bufs=2))

    # ---------------- per-head pipeline ----------------
    for h in range(H):
        tok0 = h * TB

        # Load x block (512 tokens x 512 c) and transpose to xT (c-major).
        xs = []
        for i in range(SUB):
            xt = stage.tile([128, C], f32, tag="xstage")
            nc.sync.dma_start(xt, x_d[tok0 + i * 128: tok0 + (i + 1) * 128, :])
            xs.append(xt)
        xT = headp.tile([128, CT, TB], f32r, tag="xT")
        for ct in range(CT):
            px = ps_big.tile([128, TB], f32, tag="ps")
            for i in range(SUB):
                nc.tensor.transpose(
                    px[:, i * 128:(i + 1) * 128],
                    xs[i][:, ct * 128:(ct + 1) * 128], ident)
            nc.vector.tensor_copy(xT[:, ct, :], px)

        # q,k projection for this head's tokens: qk[t, f] (f in 0:1024), +bias.
        qk = headp.tile([128, SUB, 2 * C], f32, tag="qk")
        for i in range(SUB):
            for g in range(2):
                pq = ps_big.tile([128, C], f32, tag="ps")
                for ct in range(CT):
                    nc.tensor.matmul(
                        pq,
                        xT[:, ct, i * 128:(i + 1) * 128],
                        wqkvT[:, ct, g * C:(g + 1) * C],
                        start=(ct == 0), stop=(ct == CT - 1))
                nc.vector.tensor_add(
                    qk[:, i, g * C:(g + 1) * C], pq, qk_bias[:, g * C:(g + 1) * C])

        # v projection, transposed: vT[p, ct, t] = v[h512+t, ct*128+p], +bias.
        vT = headp.tile([128, CT, TB], f32r, tag="vT")
        for ct in range(CT):
            pv = ps_big.tile([128, TB], f32, tag="ps")
            for ci in range(CT):
                nc.tensor.matmul(
                    pv,
                    wqkvT[:, ci, 2 * C + ct * 128: 2 * C + (ct + 1) * 128],
                    xT[:, ci, :],
                    start=(ci == 0), stop=(ci == CT - 1))
            nc.vector.tensor_scalar_add(vT[:, ct, :], pv, vbias[:, ct:ct + 1])

        # scores[d, e] (64x64), fp32, accumulated over 4 t-tiles x 8 s.
        sc = ps_sm.tile([64, 64], f32, tag="pss")
        nmm = SUB * H
        k = 0
        for i in range(SUB):
            for s in range(H):
                nc.tensor.matmul(
                    sc,
                    qk[:, i, s * D:(s + 1) * D],
                    qk[:, i, C + s * D: C + (s + 1) * D],
                    start=(k == 0), stop=(k == nmm - 1))
                k += 1

        # softmax over e (free axis); scale 1/sqrt(64) folded into exp.
        rmax = smallp.tile([64, 1], f32, tag="rmax")
        nc.vector.reduce_max(rmax, sc, axis=mybir.AxisListType.X)
        ebias = smallp.tile([64, 1], f32, tag="ebias")
        nc.vector.tensor_scalar_mul(ebias, rmax, -0.125)
        wexp = smallp.tile([64, 64], f32, tag="wexp")
        nc.scalar.activation(wexp, sc, mybir.ActivationFunctionType.Exp,
                             bias=ebias, scale=0.125)
        rsum = smallp.tile([64, 1], f32, tag="rsum")
        nc.vector.reduce_sum(rsum, wexp, axis=mybir.AxisListType.X)
        rrec = smallp.tile([64, 1], f32, tag="rrec")
        nc.vector.reciprocal(rrec, rsum)
        wn = smallp.tile([64, 64], f32, tag="wn")
        nc.vector.tensor_scalar_mul(wn, wexp, rrec)

        # block-diag(W^T) (128x128): diag blocks at [0:64,0:64] and [64:128,64:128].
        bd = headp.tile([128, 128], f32r, tag="bd")
        nc.vector.tensor_copy(bd, zeros128)  # fp32r memset unsupported; cast-copy zeros
        wps = ps_sm.tile([64, 64], f32, tag="pss")
        nc.tensor.transpose(wps, wn, ident[0:64, 0:64])
        nc.vector.tensor_copy(bd[0:64, 0:64], wps)
        nc.vector.tensor_copy(bd[64:128, 64:128], wps)  # partition-shifted copy

        # y: for each vT tile (two s-slices), psum rows (sl*64+d), cols t.
        j = h // 2
        pb = (h % 2) * 64
        for tau in range(CT):
            py = ps_big.tile([128, TB], f32, tag="ps")
            nc.tensor.matmul(py, bd, vT[:, tau, :], start=True, stop=True)
            # tokens m = 8t + (2*tau + sl), channel h*64+d
            nc.vector.tensor_copy(
                attnT[pb:pb + 64, j, 2 * tau::8], py[0:64, :])
            nc.vector.tensor_copy(
                attnT[pb:pb + 64, j, 2 * tau + 1::8], py[64:128, :])

    # ---------------- output projection ----------------
    for mt in range(N_TOK // 128):
        pp = ps_big.tile([128, C], f32, tag="ps")
        for j in range(CT):
            nc.tensor.matmul(
                pp, attnT[:, j, mt * 128:(mt + 1) * 128], wpT[:, j, :],
                start=(j == 0), stop=(j == CT - 1))
        ob = outp.tile([128, C], f32, tag="ob")
        nc.vector.tensor_add(ob, pp, wp_bias)
        nc.sync.dma_start(out_d[mt * 128:(mt + 1) * 128, :], ob)


def _build():
    nc = bacc.Bacc("TRN2", target_bir_lowering=False, debug=False, num_devices=8)
    x_d = nc.dram_tensor("x", [N_TOK, C], dt.float32, kind="ExternalInput").ap()
    wqkv_d = nc.dram_tensor("wqkv_w", [3 * C, C], dt.float32, kind="ExternalInput").ap()
    wqkvb_d = nc.dram_tensor("wqkv_b", [3 * C], dt.float32, kind="ExternalInput").ap()
    wp_d = nc.dram_tensor("wp_w", [C, C], dt.float32, kind="ExternalInput").ap()
    wpb_d = nc.dram_tensor("wp_b", [C], dt.float32, kind="ExternalInput").ap()
    out_d = nc.dram_tensor("out", [N_TOK, C], dt.float32, kind="ExternalOutput").ap()

    with tile.TileContext(nc) as tc:
        with ExitStack() as ctx:
            _emit(ctx, tc, out_d, x_d, wqkv_d, wqkvb_d, wp_d, wpb_d)
    nc.compile()
    return nc


def _get_nc():
    if "nc" not in _cache:
        _cache["nc"] = _build()
    return _cache["nc"]


def kernel(x, wqkv_w, wqkv_b, wp_w, wp_b, _trace=False, **_trace_kwargs):
    nc = _get_nc()
    x = np.ascontiguousarray(np.asarray(x, dtype=np.float32))
    w = {
        "wqkv_w": np.ascontiguousarray(np.asarray(wqkv_w, np.float32)),
        "wqkv_b": np.ascontiguousarray(np.asarray(wqkv_b, np.float32)),
        "wp_w": np.ascontiguousarray(np.asarray(wp_w, np.float32)),
        "wp_b": np.ascontiguousarray(np.asarray(wp_b, np.float32)),
    }
    in_maps = [dict(w, x=np.ascontiguousarray(x[i])) for i in range(8)]
    res = run_bass_kernel_spmd(nc, in_maps, list(range(8)),
                               trace=_trace, **_trace_kwargs)
    out = np.stack([r["out"] for r in res.results], axis=0).astype(np.float32)
    if _trace:
        return out, res
    return out
